# revision 3
# baseline (speedup 1.0000x reference)
"""DepthLoss kernel for 8 Trainium2 NeuronCores.

reference:
    rows/cols/d = rdepth[...,0/1/2]; mask = d>0
    vals = output[b, 0, rows, cols]
    loss = sum(mask * |vals - d|) / max(count(mask), 1)   (0 if count==0)

Strategy: data-parallel over batch (4 planes/core). Per core:
  - compute pixel index pix = r*W + c on DVE; row id = pix>>6 (int16)
  - dma_gather (SWDGE bulk gather) fetches each sample's 256B image row
  - one-hot select on DVE picks the target element out of each row
  - masked |v - d| partial sums + counts per partition -> [128, 2]
Host combines the 8 cores' partials and does the final divide.

The gather is the hard floor: 65536 descriptors/core over 4 SWDGE
queues at ~8.6ns/descriptor/queue ~= 141us.  Everything else is
arranged to hide under it:
  - idx prep for iteration i+1 runs on DVE while iteration i's gathers
    are in flight (software pipelining across the repeat-slope loop),
    so the Pool engine never waits on DVE at an iteration boundary.
  - the per-batch (row,col,depth) -> csel/mask index chain is computed
    once per iteration at 4-batch width instead of per batch.
  - csel = (cmod+1)*mask compared against iota+1 (avoids the -1 shift).

Index bookkeeping: dma_gather consumes index i from partition i%16,
column i//16 of its idx tile and writes the row to G[i%128, i//128, :].
With row-ids for sample s = 1024*q + 128*(u%8) + u//8 stored at idx
tile [q, u], the gather output G[p, jj] holds sample 128*pi(p) + jj
where pi(p) = 8*(p%16) + p//16.  Loading the per-batch rdepth with a
permuted-partition AP (partition p <- contiguous samples starting at
128*pi(p)) makes d/cmod line up with G with no cross-partition moves.
"""

import numpy as np

import concourse.bacc as bacc
import concourse.mybir as mybir
import concourse.tile as tile
from concourse import library_config
from concourse.bass_utils import run_bass_kernel_spmd

B, N, H, W = 32, 16384, 768, 1024
NCORES = 8
BPC = B // NCORES          # batches (planes) per core = 4
P = 128
PLANE = H * W              # 786432
E = 64                     # gathered row length (f32) = 256 B
RT = PLANE // E            # rows per plane table = 12288
U = N // 16                # idx columns = 1024
JJ = N // P                # samples per partition per batch = 128
HJ = JJ // 2
NCH = 16                   # gather chunks per batch
CI = N // NCH              # idxs per chunk = 1024
CJ = JJ // NCH             # dst cols per chunk = 8
CU = U // NCH              # idx tile cols per chunk = 64
F32 = mybir.dt.float32
I16 = mybir.dt.int16
I32 = mybir.dt.int32
Alu = mybir.AluOpType
AX = mybir.AxisListType


def build(n_iters=1, init_unused=True):
    nc = bacc.Bacc(
        "TRN2", target_bir_lowering=False, debug=False,
        num_swdge_queues=4,
    )

    img = nc.dram_tensor("img", [BPC * RT, E], F32, kind="ExternalInput")
    rdp = nc.dram_tensor("rdp", [BPC * N, 3], F32, kind="ExternalInput")
    out = nc.dram_tensor("out", [P, 2], F32, kind="ExternalOutput")

    with tile.TileContext(nc) as tc:
        with (
            tc.tile_pool(name="const", bufs=1) as cst,
            tc.tile_pool(name="rtp", bufs=2) as rtp,
            tc.tile_pool(name="pxp", bufs=1) as pxp,
            tc.tile_pool(name="ixp", bufs=2) as ixp,
            tc.tile_pool(name="big", bufs=2) as big,
            tc.tile_pool(name="wp", bufs=2) as wp,
            tc.tile_pool(name="sm", bufs=2) as smp,
        ):
            nc.gpsimd.load_library(library_config.mlp)
            io_i = cst.tile([P, E], I32, tag="io_i")
            nc.gpsimd.iota(io_i[:], pattern=[[1, E]], channel_multiplier=0)
            iota1 = cst.tile([P, E], F32, tag="iota1")
            nc.vector.tensor_copy(out=iota1[:], in_=io_i[:])
            nc.vector.tensor_scalar(
                out=iota1[:], in0=iota1[:], scalar1=1.0, scalar2=None,
                op0=Alu.add,
            )

            def idx_prep():
                """rt16 load + pix/row computation + idx16 tiles.

                DVE cost ~9us; emitted for iteration i+1 right after
                iteration i's gathers so it overlaps them.
                """
                rt16 = rtp.tile([P, 3 * U], F32, tag="rt16")
                if init_unused:
                    nc.vector.memset(rt16[:], 0)
                for b in range(BPC):
                    nc.sync.dma_start(
                        out=rt16[32 * b : 32 * b + 16, :],
                        in_=rdp[b * N : (b + 1) * N, :].rearrange(
                            "(q u) c -> q (u c)", q=16
                        ),
                    )
                rv16 = rt16[:].rearrange("p (u c) -> p u c", c=3)
                pix = pxp.tile([P, U], F32, tag="pix")
                nc.vector.scalar_tensor_tensor(
                    out=pix[:], in0=rv16[:, :, 0], scalar=float(W),
                    in1=rv16[:, :, 1], op0=Alu.mult, op1=Alu.add,
                )
                pixi = pxp.tile([P, U], I32, tag="pixi")
                nc.vector.tensor_copy(out=pixi[:], in_=pix[:])
                rowi = pxp.tile([P, U], I32, tag="rowi")
                nc.vector.tensor_scalar(
                    out=rowi[:], in0=pixi[:], scalar1=6, scalar2=None,
                    op0=Alu.arith_shift_right,
                )
                idxs = []
                for b in range(BPC):
                    qs = slice(32 * b, 32 * b + 16)
                    # int16 row ids: idx16[q, u] = rowi[32b+q, 128*(u%8)+u//8],
                    # replicated across all 8 gpsimd-core stripes
                    idx16 = ixp.tile([P, U], I16, tag=f"idx16_{b}")
                    nc.vector.tensor_copy(
                        out=idx16[0:16, :]
                        .rearrange("q (a e) -> q a e", e=8)
                        .transpose([0, 2, 1]),
                        in_=rowi[qs, :].rearrange("q (e a) -> q e a", e=8),
                    )
                    nc.sync.dma_start(out=idx16[16:32, :], in_=idx16[0:16, :])
                    nc.sync.dma_start(out=idx16[32:64, :], in_=idx16[0:32, :])
                    nc.sync.dma_start(out=idx16[64:128, :], in_=idx16[0:64, :])
                    idxs.append(idx16)
                return idxs

            idxs = idx_prep()
            for it in range(n_iters):
                # per-batch rdepth in gather layout, all batches wide:
                # rtbw[p, b, t, c] = rdepth[b, 128*pi(p)+t, c]
                rtbw = smp.tile([P, BPC * 3 * JJ], F32, tag="rtbw")
                for b in range(BPC):
                    src = bacc.bass.AP(
                        rdp,
                        b * N * 3,
                        [[3 * JJ, 8], [8 * 3 * JJ, 16], [1, 3 * JJ]],
                    )
                    nc.sync.dma_start(
                        out=rtbw[:, b * 3 * JJ : (b + 1) * 3 * JJ], in_=src
                    )

                # gathers: G[p, jj, :] = img row of sample 128*pi(p)+jj
                # (chunked so each SWDGE op fits the descriptor ring and
                #  rotates the 4 queues)
                gs = []
                for b in range(BPC):
                    g = big.tile([P, JJ * E], F32, tag="G")
                    g3 = g[:].rearrange("p (j e) -> p j e", e=E)
                    gs.append(g3)
                    for k in range(NCH):
                        nc.gpsimd.dma_gather(
                            g3[:, k * CJ : (k + 1) * CJ, :],
                            img[b * RT : (b + 1) * RT, :],
                            idxs[b][:, k * CU : (k + 1) * CU],
                            CI,
                            CI,
                            E,
                            single_packet=False,
                            queue_num=k % 4,
                        )

                # idx prep for the NEXT iteration: overlaps this
                # iteration's gathers on DVE
                if it + 1 < n_iters:
                    idxs = idx_prep()

                # wide select-index chain (all 4 batches at once)
                rv = rtbw[:].rearrange("p (b t c) -> p b t c", b=BPC, c=3)
                pixw = smp.tile([P, BPC * JJ], F32, tag="pixw")
                pw3 = pixw[:].rearrange("p (b t) -> p b t", b=BPC)
                nc.vector.scalar_tensor_tensor(
                    out=pw3, in0=rv[:, :, :, 0], scalar=float(W),
                    in1=rv[:, :, :, 1], op0=Alu.mult, op1=Alu.add,
                )
                pixwi = smp.tile([P, BPC * JJ], I32, tag="pixwi")
                nc.vector.tensor_copy(out=pixwi[:], in_=pixw[:])
                cmw = smp.tile([P, BPC * JJ], I32, tag="cmw")
                nc.vector.tensor_scalar(
                    out=cmw[:], in0=pixwi[:], scalar1=E - 1, scalar2=None,
                    op0=Alu.bitwise_and,
                )
                cselw = smp.tile([P, BPC * JJ], F32, tag="cselw")
                nc.vector.tensor_copy(out=cselw[:], in_=cmw[:])
                mselw = smp.tile([P, BPC * JJ], F32, tag="mselw")
                nc.vector.tensor_scalar(
                    out=mselw[:], in0=rv[:, :, :, 2], scalar1=0.0,
                    scalar2=None, op0=Alu.is_gt,
                )
                # csel = (cmod+1)*mask; invalid samples select nothing
                # (compared against iota+1 which is >= 1)
                nc.vector.scalar_tensor_tensor(
                    out=cselw[:], in0=cselw[:], scalar=1.0, in1=mselw[:],
                    op0=Alu.add, op1=Alu.mult,
                )
                cs3 = cselw[:].rearrange("p (b t) -> p b t", b=BPC)
                ms3 = mselw[:].rearrange("p (b t) -> p b t", b=BPC)

                lc2 = smp.tile([P, BPC], F32, tag="lc2")
                for b in range(BPC):
                    # one-hot select: W = (iota+1 == csel); v = sum(G*W)
                    # split into jj-halves so each half's mul starts as
                    # soon as its 8 gather chunks land (subtile deps)
                    vsel = smp.tile([P, JJ], F32, tag="vsel")
                    for h in range(2):
                        js = slice(h * HJ, (h + 1) * HJ)
                        w = wp.tile([P, HJ * E], F32, tag="W")
                        w3 = w[:].rearrange("p (j e) -> p j e", e=E)
                        nc.vector.tensor_tensor(
                            out=w3,
                            in0=iota1[:].unsqueeze(1).to_broadcast(
                                [P, HJ, E]
                            ),
                            in1=cs3[:, b, js].unsqueeze(2).to_broadcast(
                                [P, HJ, E]
                            ),
                            op=Alu.is_equal,
                        )
                        nc.vector.tensor_tensor(
                            out=w3, in0=gs[b][:, js, :], in1=w3, op=Alu.mult
                        )
                        nc.vector.tensor_reduce(
                            out=vsel[:, js], in_=w3, axis=AX.X, op=Alu.add
                        )
                    # masked |v - d|
                    diff = smp.tile([P, JJ], F32, tag="diff")
                    nc.vector.tensor_tensor(
                        out=diff[:], in0=vsel[:], in1=rv[:, b, :, 2],
                        op=Alu.subtract,
                    )
                    nc.vector.tensor_tensor(
                        out=diff[:], in0=diff[:], in1=ms3[:, b, :],
                        op=Alu.mult,
                    )
                    nc.vector.tensor_reduce(
                        out=lc2[:, b : b + 1], in_=diff[:], axis=AX.X,
                        op=Alu.add, apply_absolute_value=True,
                    )

                losscnt = smp.tile([P, 2], F32, tag="losscnt")
                nc.vector.tensor_reduce(
                    out=losscnt[:, 0:1], in_=lc2[:], axis=AX.X, op=Alu.add
                )
                nc.vector.tensor_reduce(
                    out=losscnt[:, 1:2], in_=mselw[:], axis=AX.X, op=Alu.add
                )
                nc.sync.dma_start(out=out[:, :], in_=losscnt[:])

    nc.compile()
    return nc


_NC = None


def _get_nc():
    global _NC
    if _NC is None:
        _NC = build(init_unused=False)
    return _NC


def make_in_maps(output, rdepth):
    in_maps = []
    for c in range(NCORES):
        sl = slice(c * BPC, (c + 1) * BPC)
        img_c = np.ascontiguousarray(
            output[sl, 0], dtype=np.float32
        ).reshape(BPC * RT, E)
        rdp_c = np.ascontiguousarray(
            rdepth[sl], dtype=np.float32
        ).reshape(BPC * N, 3)
        in_maps.append({"img": img_c, "rdp": rdp_c})
    return in_maps


def combine(results):
    partials = np.stack([r["out"] for r in results])  # [8, 128, 2]
    loss = partials[..., 0].astype(np.float64).sum()
    cnt = partials[..., 1].astype(np.float64).sum()
    val = loss / max(cnt, 1.0) if cnt > 0 else 0.0
    return np.asarray(val, dtype=np.float32)


def run(output, rdepth, **kw):
    res = run_bass_kernel_spmd(
        _get_nc(), make_in_maps(output, rdepth), list(range(NCORES)), **kw
    )
    return combine(res.results), res


def kernel(output, rdepth):
    return run(output, rdepth)[0]


# revision 28
# speedup vs baseline: 1.1677x; 1.1677x over previous
"""DepthLoss kernel for 8 Trainium2 NeuronCores.

reference:
    rows/cols/d = rdepth[...,0/1/2]; mask = d>0
    vals = output[b, 0, rows, cols]
    loss = sum(mask * |vals - d|) / max(count(mask), 1)   (0 if count==0)

Strategy: data-parallel over batch (4 planes/core). Per core:
  - compute pixel index pix = r*W + c on DVE; row id = pix>>6 (int16)
  - dma_gather (SWDGE bulk gather) fetches each sample's 256B image row
  - one-hot select on DVE picks the target element out of each row
  - masked |v - d| partial sums + counts per partition -> [128, 2]
Host combines the 8 cores' partials and does the final divide.

The gather is the hard floor: 65536 descriptors/core over 4 SWDGE
queues at ~8.6ns/descriptor/queue ~= 141us.  Everything else is
arranged to hide under it:
  - idx prep for iteration i+1 runs on DVE while iteration i's gathers
    are in flight (software pipelining across the repeat-slope loop),
    so the Pool engine never waits on DVE at an iteration boundary.
  - the per-batch (row,col,depth) -> csel/mask index chain is computed
    once per iteration at 4-batch width instead of per batch.
  - csel = (cmod+1)*mask compared against iota+1 (avoids the -1 shift).

Index bookkeeping: dma_gather consumes index i from partition i%16,
column i//16 of its idx tile and writes the row to G[i%128, i//128, :].
With row-ids for sample s = 1024*q + 128*(u%8) + u//8 stored at idx
tile [q, u], the gather output G[p, jj] holds sample 128*pi(p) + jj
where pi(p) = 8*(p%16) + p//16.  Loading the per-batch rdepth with a
permuted-partition AP (partition p <- contiguous samples starting at
128*pi(p)) makes d/cmod line up with G with no cross-partition moves.
"""

import numpy as np

import concourse.bacc as bacc
import concourse.mybir as mybir
import concourse.tile as tile
from concourse import library_config
from concourse.bass_utils import run_bass_kernel_spmd

B, N, H, W = 32, 16384, 768, 1024
NCORES = 8
BPC = B // NCORES          # batches (planes) per core = 4
P = 128
PLANE = H * W              # 786432
E = 64                     # gathered row length (f32) = 256 B
RT = PLANE // E            # rows per plane table = 12288
U = N // 16                # idx columns = 1024
JJ = N // P                # samples per partition per batch = 128
HJ = JJ // 2
NCH = 8                    # gather chunks per batch
CI = N // NCH              # idxs per chunk = 1024
CJ = JJ // NCH             # dst cols per chunk = 8
CU = U // NCH              # idx tile cols per chunk = 64
F32 = mybir.dt.float32
BF16 = mybir.dt.bfloat16
I16 = mybir.dt.int16
I32 = mybir.dt.int32
Alu = mybir.AluOpType
AX = mybir.AxisListType


def build(n_iters=1, init_unused=True, nch=NCH, do_select=True, pure=False, rtbw_only=False, gbufs=4, wbufs=2, act_idx=True, wide_tail=False, sel_splits=2, half_sel=False, act_tail=True):
    nc = bacc.Bacc(
        "TRN2", target_bir_lowering=False, debug=False,
        num_swdge_queues=4,
    )

    img = nc.dram_tensor("img", [BPC * RT, E], F32, kind="ExternalInput")
    rdp = nc.dram_tensor("rdp", [BPC * N, 3], F32, kind="ExternalInput")
    out = nc.dram_tensor("out", [P, 2], F32, kind="ExternalOutput")

    with tile.TileContext(nc) as tc:
        with (
            tc.tile_pool(name="const", bufs=1) as cst,
            tc.tile_pool(name="rtp", bufs=2) as rtp,
            tc.tile_pool(name="pxp", bufs=1) as pxp,
            tc.tile_pool(name="ixp", bufs=2) as ixp,
            tc.tile_pool(name="big", bufs=gbufs) as big,
            tc.tile_pool(name="wp", bufs=wbufs) as wp,
            tc.tile_pool(name="sm", bufs=2) as smp,
        ):
            nc.gpsimd.load_library(library_config.mlp)
            io_i = cst.tile([P, E], I32, tag="io_i")
            nc.gpsimd.iota(io_i[:], pattern=[[1, E]], channel_multiplier=0)
            iota1 = cst.tile([P, E], F32, tag="iota1")
            nc.vector.tensor_copy(out=iota1[:], in_=io_i[:])
            nc.vector.tensor_scalar(
                out=iota1[:], in0=iota1[:], scalar1=1.0, scalar2=None,
                op0=Alu.add,
            )

            def idx_prep():
                """rt16 load + pix/row computation + idx16 tiles.

                DVE cost ~9us; emitted for iteration i+1 right after
                iteration i's gathers so it overlaps them.  Returns the
                rt16 tile too: the select phase of the SAME iteration
                re-reads it (SBUF->SBUF) to build the gather-layout
                rdepth view instead of re-reading HBM, which would
                contend with the gather stream.
                """
                rt16 = rtp.tile([P, 3 * U], F32, tag="rt16")
                if init_unused:
                    nc.vector.memset(rt16[:], 0)
                for b in range(BPC):
                    nc.sync.dma_start(
                        out=rt16[32 * b : 32 * b + 16, :],
                        in_=rdp[b * N : (b + 1) * N, :].rearrange(
                            "(q u) c -> q (u c)", q=16
                        ),
                    )
                rv16 = rt16[:].rearrange("p (u c) -> p u c", c=3)
                pix = pxp.tile([P, U], F32, tag="pix")
                nc.vector.scalar_tensor_tensor(
                    out=pix[:], in0=rv16[:, :, 0], scalar=float(W),
                    in1=rv16[:, :, 1], op0=Alu.mult, op1=Alu.add,
                )
                pixi = pxp.tile([P, U], I32, tag="pixi")
                if act_idx:
                    nc.scalar.activation(
                        out=pixi[:], in_=pix[:],
                        func=mybir.ActivationFunctionType.Copy,
                    )
                else:
                    nc.vector.tensor_copy(out=pixi[:], in_=pix[:])
                rowi = pxp.tile([P, U], I32, tag="rowi")
                nc.vector.tensor_scalar(
                    out=rowi[:], in0=pixi[:], scalar1=6, scalar2=None,
                    op0=Alu.arith_shift_right,
                )
                idxs = []
                for b in range(BPC):
                    qs = slice(32 * b, 32 * b + 16)
                    # int16 row ids: idx16[q, u] = rowi[32b+q, 128*(u%8)+u//8],
                    # replicated across all 8 gpsimd-core stripes
                    idx16 = ixp.tile([P, U], I16, tag=f"idx16_{b}")
                    oap = (
                        idx16[0:16, :]
                        .rearrange("q (a e) -> q a e", e=8)
                        .transpose([0, 2, 1])
                    )
                    iap = rowi[qs, :].rearrange("q (e a) -> q e a", e=8)
                    if act_idx:
                        nc.scalar.activation(
                            out=oap, in_=iap,
                            func=mybir.ActivationFunctionType.Copy,
                        )
                    else:
                        nc.vector.tensor_copy(out=oap, in_=iap)
                    nc.sync.dma_start(out=idx16[16:32, :], in_=idx16[0:16, :])
                    nc.sync.dma_start(out=idx16[32:64, :], in_=idx16[0:32, :])
                    nc.sync.dma_start(out=idx16[64:128, :], in_=idx16[0:64, :])
                    idxs.append(idx16)
                return idxs, rt16

            idxs, rt16c = idx_prep()
            for it in range(n_iters):
                # per-batch rdepth in gather layout, all batches wide:
                # rtbw[16a+q, b, t, c] = rdepth[b, 1024q+128a+t, c]
                #                      = rt16[32b+q, 384a+3t+c]
                # (pure SBUF->SBUF relayout of rt16; no HBM traffic, so
                #  it does not disturb the gather stream.  Issued from
                #  the otherwise-idle ACT queue.)
                rtbw = smp.tile([P, BPC * 3 * JJ], F32, tag="rtbw")
                if pure and not rtbw_only:
                    nc.vector.memset(rtbw[:], 0)
                for b in range(BPC if (not pure or rtbw_only) else 0):
                    for a in range(8):
                        nc.scalar.dma_start(
                            out=rtbw[
                                16 * a : 16 * (a + 1),
                                b * 3 * JJ : (b + 1) * 3 * JJ,
                            ],
                            in_=rt16c[
                                32 * b : 32 * b + 16,
                                384 * a : 384 * (a + 1),
                            ],
                        )

                # gathers: G[p, jj, :] = img row of sample 128*pi(p)+jj
                # (chunked so each SWDGE op fits the descriptor ring and
                #  rotates the 4 queues)
                gs = {}
                ci = N // nch
                cj = JJ // nch
                cu = U // nch
                hk = nch // 2          # chunks per half
                for b in range(BPC):
                    for h in range(2):
                        g = big.tile([P, HJ * E], F32, tag="G")
                        g3 = g[:].rearrange("p (j e) -> p j e", e=E)
                        gs[(b, h)] = g3
                        for kk in range(hk):
                            k = h * hk + kk
                            nc.gpsimd.dma_gather(
                                g3[:, kk * cj : (kk + 1) * cj, :],
                                img[b * RT : (b + 1) * RT, :],
                                idxs[b][:, k * cu : (k + 1) * cu],
                                ci,
                                ci,
                                E,
                                single_packet=False,
                                queue_num=k % 4,
                            )

                # idx prep for the NEXT iteration: overlaps this
                # iteration's gathers on DVE
                if it + 1 < n_iters:
                    idxs, rt16c = idx_prep()

                # wide select-index chain (all 4 batches at once)
                rv = rtbw[:].rearrange("p (b t c) -> p b t c", b=BPC, c=3)
                pixw = smp.tile([P, BPC * JJ], F32, tag="pixw")
                pw3 = pixw[:].rearrange("p (b t) -> p b t", b=BPC)
                nc.vector.scalar_tensor_tensor(
                    out=pw3, in0=rv[:, :, :, 0], scalar=float(W),
                    in1=rv[:, :, :, 1], op0=Alu.mult, op1=Alu.add,
                )
                pixwi = smp.tile([P, BPC * JJ], I32, tag="pixwi")
                nc.vector.tensor_copy(out=pixwi[:], in_=pixw[:])
                cmw = smp.tile([P, BPC * JJ], I32, tag="cmw")
                nc.vector.tensor_scalar(
                    out=cmw[:], in0=pixwi[:], scalar1=E - 1, scalar2=None,
                    op0=Alu.bitwise_and,
                )
                cselw = smp.tile([P, BPC * JJ], F32, tag="cselw")
                nc.vector.tensor_copy(out=cselw[:], in_=cmw[:])
                mselw = smp.tile([P, BPC * JJ], F32, tag="mselw")
                nc.vector.tensor_scalar(
                    out=mselw[:], in0=rv[:, :, :, 2], scalar1=0.0,
                    scalar2=None, op0=Alu.is_gt,
                )
                # csel = (cmod+1)*mask; invalid samples select nothing
                # (compared against iota+1 which is >= 1)
                nc.vector.scalar_tensor_tensor(
                    out=cselw[:], in0=cselw[:], scalar=1.0, in1=mselw[:],
                    op0=Alu.add, op1=Alu.mult,
                )
                cs3 = cselw[:].rearrange("p (b t) -> p b t", b=BPC)
                ms3 = mselw[:].rearrange("p (b t) -> p b t", b=BPC)

                vsels = []
                sj = JJ // sel_splits       # samples per select split
                sh = sel_splits // 2        # splits per gather half
                for b in range(BPC if do_select else 0):
                    vsel = smp.tile([P, JJ], F32, tag="vsel")
                    vsels.append(vsel)
                    if half_sel:
                        nc.vector.memset(vsel[:, HJ:], 0)
                    # one-hot select: W = (iota+1 == csel); v = sum(G*W)
                    # split so each piece's mul starts as soon as its
                    # gather chunks land (subtile deps)
                    for q in range(sel_splits if not half_sel else 1):
                        js = slice(q * sj, (q + 1) * sj)
                        h = q // sh
                        jg = slice((q % sh) * sj, (q % sh + 1) * sj)
                        w = wp.tile([P, sj * E], F32, tag="W")
                        w3 = w[:].rearrange("p (j e) -> p j e", e=E)
                        nc.vector.tensor_tensor(
                            out=w3,
                            in0=iota1[:].unsqueeze(1).to_broadcast(
                                [P, sj, E]
                            ),
                            in1=cs3[:, b, js].unsqueeze(2).to_broadcast(
                                [P, sj, E]
                            ),
                            op=Alu.is_equal,
                        )
                        nc.vector.tensor_tensor(
                            out=w3, in0=gs[(b, h)][:, jg, :], in1=w3,
                            op=Alu.mult,
                        )
                        nc.vector.tensor_reduce(
                            out=vsels[b][:, js], in_=w3, axis=AX.X, op=Alu.add
                        )
                losscnt = smp.tile([P, 2], F32, tag="losscnt")
                lc2 = smp.tile([P, BPC], F32, tag="lc2")
                if not do_select:
                    nc.vector.memset(lc2[:], 0)
                for b in range(BPC if do_select else 0):
                    # masked |v - d|
                    diff = smp.tile([P, JJ], F32, tag="diff")
                    nc.vector.tensor_tensor(
                        out=diff[:], in0=vsels[b][:], in1=rv[:, b, :, 2],
                        op=Alu.subtract,
                    )
                    nc.vector.tensor_tensor(
                        out=diff[:], in0=diff[:], in1=ms3[:, b, :],
                        op=Alu.mult,
                    )
                    if act_tail:
                        # |.| + free-dim accumulate in one ACT op
                        absd = smp.tile([P, JJ], F32, tag="absd")
                        nc.scalar.activation(
                            out=absd[:], in_=diff[:],
                            func=mybir.ActivationFunctionType.Abs,
                            accum_out=lc2[:, b : b + 1],
                        )
                    else:
                        nc.vector.tensor_reduce(
                            out=lc2[:, b : b + 1], in_=diff[:], axis=AX.X,
                            op=Alu.add, apply_absolute_value=True,
                        )
                if act_tail:
                    lsum = smp.tile([P, BPC], F32, tag="lsum")
                    nc.scalar.activation(
                        out=lsum[:], in_=lc2[:],
                        func=mybir.ActivationFunctionType.Copy,
                        accum_out=losscnt[:, 0:1],
                    )
                else:
                    nc.vector.tensor_reduce(
                        out=losscnt[:, 0:1], in_=lc2[:], axis=AX.X, op=Alu.add
                    )
                nc.vector.tensor_reduce(
                    out=losscnt[:, 1:2], in_=mselw[:], axis=AX.X, op=Alu.add
                )
                nc.sync.dma_start(out=out[:, :], in_=losscnt[:])

    nc.compile()
    return nc


_NC = None


def _get_nc():
    global _NC
    if _NC is None:
        _NC = build(init_unused=False)
    return _NC


def make_in_maps(output, rdepth):
    in_maps = []
    for c in range(NCORES):
        sl = slice(c * BPC, (c + 1) * BPC)
        img_c = np.ascontiguousarray(
            output[sl, 0], dtype=np.float32
        ).reshape(BPC * RT, E)
        rdp_c = np.ascontiguousarray(
            rdepth[sl], dtype=np.float32
        ).reshape(BPC * N, 3)
        in_maps.append({"img": img_c, "rdp": rdp_c})
    return in_maps


def combine(results):
    partials = np.stack([r["out"] for r in results])  # [8, 128, 2]
    loss = partials[..., 0].astype(np.float64).sum()
    cnt = partials[..., 1].astype(np.float64).sum()
    val = loss / max(cnt, 1.0) if cnt > 0 else 0.0
    return np.asarray(val, dtype=np.float32)


def run(output, rdepth, **kw):
    res = run_bass_kernel_spmd(
        _get_nc(), make_in_maps(output, rdepth), list(range(NCORES)), **kw
    )
    return combine(res.results), res


def kernel(output, rdepth):
    return run(output, rdepth)[0]


# revision 29
# speedup vs baseline: 1.2573x; 1.0767x over previous
"""DepthLoss kernel for 8 Trainium2 NeuronCores.

reference:
    rows/cols/d = rdepth[...,0/1/2]; mask = d>0
    vals = output[b, 0, rows, cols]
    loss = sum(mask * |vals - d|) / max(count(mask), 1)   (0 if count==0)

Strategy: data-parallel over batch (4 planes/core). Per core:
  - compute pixel index pix = r*W + c on DVE; row id = pix>>6 (int16)
  - dma_gather (SWDGE bulk gather) fetches each sample's 256B image row
  - one-hot select on DVE picks the target element out of each row
  - masked |v - d| partial sums + counts per partition -> [128, 2]
Host combines the 8 cores' partials and does the final divide.

The gather is the hard floor: 65536 descriptors/core over 4 SWDGE
queues at ~8.6ns/descriptor/queue ~= 141us.  Everything else is
arranged to hide under it:
  - idx prep for iteration i+1 runs on DVE while iteration i's gathers
    are in flight (software pipelining across the repeat-slope loop),
    so the Pool engine never waits on DVE at an iteration boundary.
  - the per-batch (row,col,depth) -> csel/mask index chain is computed
    once per iteration at 4-batch width instead of per batch.
  - csel = (cmod+1)*mask compared against iota+1 (avoids the -1 shift).

Index bookkeeping: dma_gather consumes index i from partition i%16,
column i//16 of its idx tile and writes the row to G[i%128, i//128, :].
With row-ids for sample s = 1024*q + 128*(u%8) + u//8 stored at idx
tile [q, u], the gather output G[p, jj] holds sample 128*pi(p) + jj
where pi(p) = 8*(p%16) + p//16.  Loading the per-batch rdepth with a
permuted-partition AP (partition p <- contiguous samples starting at
128*pi(p)) makes d/cmod line up with G with no cross-partition moves.
"""

import numpy as np

import concourse.bacc as bacc
import concourse.mybir as mybir
import concourse.tile as tile
from concourse import library_config
from concourse.bass_utils import run_bass_kernel_spmd

B, N, H, W = 32, 16384, 768, 1024
NCORES = 8
BPC = B // NCORES          # batches (planes) per core = 4
P = 128
PLANE = H * W              # 786432
E = 64                     # gathered row length (f32) = 256 B
RT = PLANE // E            # rows per plane table = 12288
U = N // 16                # idx columns = 1024
JJ = N // P                # samples per partition per batch = 128
HJ = JJ // 2
NCH = 8                    # gather chunks per batch
CI = N // NCH              # idxs per chunk = 1024
CJ = JJ // NCH             # dst cols per chunk = 8
CU = U // NCH              # idx tile cols per chunk = 64
F32 = mybir.dt.float32
BF16 = mybir.dt.bfloat16
I16 = mybir.dt.int16
I32 = mybir.dt.int32
Alu = mybir.AluOpType
AX = mybir.AxisListType


def build(n_iters=1, init_unused=True, nch=NCH, do_select=True, pure=False, rtbw_only=False, gbufs=4, wbufs=2, act_idx=True, wide_tail=False, sel_splits=2, half_sel=False, act_tail=True):
    nc = bacc.Bacc(
        "TRN2", target_bir_lowering=False, debug=False,
        num_swdge_queues=4,
    )

    img = nc.dram_tensor("img", [BPC * RT, E], F32, kind="ExternalInput")
    rdp = nc.dram_tensor("rdp", [BPC * N, 3], F32, kind="ExternalInput")
    out = nc.dram_tensor("out", [P, 2], F32, kind="ExternalOutput")

    with tile.TileContext(nc) as tc:
        with (
            tc.tile_pool(name="const", bufs=1) as cst,
            tc.tile_pool(name="rtp", bufs=2) as rtp,
            tc.tile_pool(name="pxp", bufs=1) as pxp,
            tc.tile_pool(name="ixp", bufs=2) as ixp,
            tc.tile_pool(name="big", bufs=gbufs) as big,
            tc.tile_pool(name="wp", bufs=wbufs) as wp,
            tc.tile_pool(name="sm", bufs=2) as smp,
        ):
            nc.gpsimd.load_library(library_config.mlp)
            io_i = cst.tile([P, E], I32, tag="io_i")
            nc.gpsimd.iota(io_i[:], pattern=[[1, E]], channel_multiplier=0)
            iota1 = cst.tile([P, E], F32, tag="iota1")
            nc.vector.tensor_copy(out=iota1[:], in_=io_i[:])
            nc.vector.tensor_scalar(
                out=iota1[:], in0=iota1[:], scalar1=1.0, scalar2=None,
                op0=Alu.add,
            )

            def idx_prep():
                """rt16 load + pix/row computation + idx16 tiles.

                DVE cost ~9us; emitted for iteration i+1 right after
                iteration i's gathers so it overlaps them.  Returns the
                rt16 tile too: the select phase of the SAME iteration
                re-reads it (SBUF->SBUF) to build the gather-layout
                rdepth view instead of re-reading HBM, which would
                contend with the gather stream.
                """
                rt16 = rtp.tile([P, 3 * U], F32, tag="rt16")
                if init_unused:
                    nc.vector.memset(rt16[:], 0)
                for b in range(BPC):
                    nc.sync.dma_start(
                        out=rt16[32 * b : 32 * b + 16, :],
                        in_=rdp[b * N : (b + 1) * N, :].rearrange(
                            "(q u) c -> q (u c)", q=16
                        ),
                    )
                rv16 = rt16[:].rearrange("p (u c) -> p u c", c=3)
                pix = pxp.tile([P, U], F32, tag="pix")
                nc.vector.scalar_tensor_tensor(
                    out=pix[:], in0=rv16[:, :, 0], scalar=float(W),
                    in1=rv16[:, :, 1], op0=Alu.mult, op1=Alu.add,
                )
                pixi = pxp.tile([P, U], I32, tag="pixi")
                if act_idx:
                    nc.scalar.activation(
                        out=pixi[:], in_=pix[:],
                        func=mybir.ActivationFunctionType.Copy,
                    )
                else:
                    nc.vector.tensor_copy(out=pixi[:], in_=pix[:])
                rowi = pxp.tile([P, U], I32, tag="rowi")
                nc.vector.tensor_scalar(
                    out=rowi[:], in0=pixi[:], scalar1=6, scalar2=None,
                    op0=Alu.arith_shift_right,
                )
                idxs = []
                for b in range(BPC):
                    qs = slice(32 * b, 32 * b + 16)
                    # int16 row ids: idx16[q, u] = rowi[32b+q, 128*(u%8)+u//8],
                    # replicated across all 8 gpsimd-core stripes
                    idx16 = ixp.tile([P, U], I16, tag=f"idx16_{b}")
                    oap = (
                        idx16[0:16, :]
                        .rearrange("q (a e) -> q a e", e=8)
                        .transpose([0, 2, 1])
                    )
                    iap = rowi[qs, :].rearrange("q (e a) -> q e a", e=8)
                    if act_idx:
                        nc.scalar.activation(
                            out=oap, in_=iap,
                            func=mybir.ActivationFunctionType.Copy,
                        )
                    else:
                        nc.vector.tensor_copy(out=oap, in_=iap)
                    nc.sync.dma_start(out=idx16[16:32, :], in_=idx16[0:16, :])
                    nc.sync.dma_start(out=idx16[32:64, :], in_=idx16[0:32, :])
                    nc.sync.dma_start(out=idx16[64:128, :], in_=idx16[0:64, :])
                    idxs.append(idx16)
                return idxs, rt16

            idxs, rt16c = idx_prep()
            for it in range(n_iters):
                # per-batch rdepth in gather layout, all batches wide:
                # rtbw[16a+q, b, t, c] = rdepth[b, 1024q+128a+t, c]
                #                      = rt16[32b+q, 384a+3t+c]
                # (pure SBUF->SBUF relayout of rt16; no HBM traffic, so
                #  it does not disturb the gather stream.  Issued from
                #  the otherwise-idle ACT queue.)
                rtbw = smp.tile([P, BPC * 3 * JJ], F32, tag="rtbw")
                if pure and not rtbw_only:
                    nc.vector.memset(rtbw[:], 0)
                for b in range(BPC if (not pure or rtbw_only) else 0):
                    for a in range(8):
                        nc.scalar.dma_start(
                            out=rtbw[
                                16 * a : 16 * (a + 1),
                                b * 3 * JJ : (b + 1) * 3 * JJ,
                            ],
                            in_=rt16c[
                                32 * b : 32 * b + 16,
                                384 * a : 384 * (a + 1),
                            ],
                        )

                # gathers: G[p, jj, :] = img row of sample 128*pi(p)+jj
                # (chunked so each SWDGE op fits the descriptor ring and
                #  rotates the 4 queues)
                gs = {}
                ci = N // nch
                cj = JJ // nch
                cu = U // nch
                hk = nch // 2          # chunks per half
                for b in range(BPC):
                    for h in range(2):
                        g = big.tile([P, HJ * E], F32, tag="G")
                        g3 = g[:].rearrange("p (j e) -> p j e", e=E)
                        gs[(b, h)] = g3
                        for kk in range(hk):
                            k = h * hk + kk
                            nc.gpsimd.dma_gather(
                                g3[:, kk * cj : (kk + 1) * cj, :],
                                img[b * RT : (b + 1) * RT, :],
                                idxs[b][:, k * cu : (k + 1) * cu],
                                ci,
                                ci,
                                E,
                                single_packet=False,
                                queue_num=k % 4,
                            )

                # wide select-index chain (all 4 batches at once);
                # emitted BEFORE next-iter idx prep so the first select
                # is never gated on the DVE finishing idx work
                rv = rtbw[:].rearrange("p (b t c) -> p b t c", b=BPC, c=3)
                pixw = smp.tile([P, BPC * JJ], F32, tag="pixw")
                pw3 = pixw[:].rearrange("p (b t) -> p b t", b=BPC)
                nc.vector.scalar_tensor_tensor(
                    out=pw3, in0=rv[:, :, :, 0], scalar=float(W),
                    in1=rv[:, :, :, 1], op0=Alu.mult, op1=Alu.add,
                )
                pixwi = smp.tile([P, BPC * JJ], I32, tag="pixwi")
                nc.vector.tensor_copy(out=pixwi[:], in_=pixw[:])
                cmw = smp.tile([P, BPC * JJ], I32, tag="cmw")
                nc.vector.tensor_scalar(
                    out=cmw[:], in0=pixwi[:], scalar1=E - 1, scalar2=None,
                    op0=Alu.bitwise_and,
                )
                cselw = smp.tile([P, BPC * JJ], F32, tag="cselw")
                nc.scalar.activation(
                    out=cselw[:], in_=cmw[:],
                    func=mybir.ActivationFunctionType.Copy,
                )
                mselw = smp.tile([P, BPC * JJ], F32, tag="mselw")
                mw3 = mselw[:].rearrange("p (b t) -> p b t", b=BPC)
                nc.scalar.activation(
                    out=mw3, in_=rv[:, :, :, 2],
                    func=mybir.ActivationFunctionType.Sign,
                )
                nc.scalar.activation(
                    out=mselw[:], in_=mselw[:],
                    func=mybir.ActivationFunctionType.Relu,
                )
                # csel = (cmod+1)*mask; invalid samples select nothing
                # (compared against iota+1 which is >= 1)
                nc.vector.scalar_tensor_tensor(
                    out=cselw[:], in0=cselw[:], scalar=1.0, in1=mselw[:],
                    op0=Alu.add, op1=Alu.mult,
                )
                cs3 = cselw[:].rearrange("p (b t) -> p b t", b=BPC)
                ms3 = mselw[:].rearrange("p (b t) -> p b t", b=BPC)

                # idx prep for the NEXT iteration: overlaps this
                # iteration's gathers
                if it + 1 < n_iters:
                    idxs, rt16c = idx_prep()

                vsels = []
                sj = JJ // sel_splits       # samples per select split
                sh = sel_splits // 2        # splits per gather half
                for b in range(BPC if do_select else 0):
                    vsel = smp.tile([P, JJ], F32, tag="vsel")
                    vsels.append(vsel)
                    if half_sel:
                        nc.vector.memset(vsel[:, HJ:], 0)
                    # one-hot select: W = (iota+1 == csel); v = sum(G*W)
                    # split so each piece's mul starts as soon as its
                    # gather chunks land (subtile deps)
                    for q in range(sel_splits if not half_sel else 1):
                        js = slice(q * sj, (q + 1) * sj)
                        h = q // sh
                        jg = slice((q % sh) * sj, (q % sh + 1) * sj)
                        w = wp.tile([P, sj * E], F32, tag="W")
                        w3 = w[:].rearrange("p (j e) -> p j e", e=E)
                        nc.vector.tensor_tensor(
                            out=w3,
                            in0=iota1[:].unsqueeze(1).to_broadcast(
                                [P, sj, E]
                            ),
                            in1=cs3[:, b, js].unsqueeze(2).to_broadcast(
                                [P, sj, E]
                            ),
                            op=Alu.is_equal,
                        )
                        nc.vector.tensor_tensor(
                            out=w3, in0=gs[(b, h)][:, jg, :], in1=w3,
                            op=Alu.mult,
                        )
                        nc.vector.tensor_reduce(
                            out=vsels[b][:, js], in_=w3, axis=AX.X, op=Alu.add
                        )
                losscnt = smp.tile([P, 2], F32, tag="losscnt")
                lc2 = smp.tile([P, BPC], F32, tag="lc2")
                if not do_select:
                    nc.vector.memset(lc2[:], 0)
                for b in range(BPC if do_select else 0):
                    # masked |v - d|
                    diff = smp.tile([P, JJ], F32, tag="diff")
                    nc.vector.tensor_tensor(
                        out=diff[:], in0=vsels[b][:], in1=rv[:, b, :, 2],
                        op=Alu.subtract,
                    )
                    nc.vector.tensor_tensor(
                        out=diff[:], in0=diff[:], in1=ms3[:, b, :],
                        op=Alu.mult,
                    )
                    if act_tail:
                        # |.| + free-dim accumulate in one ACT op
                        absd = smp.tile([P, JJ], F32, tag="absd")
                        nc.scalar.activation(
                            out=absd[:], in_=diff[:],
                            func=mybir.ActivationFunctionType.Abs,
                            accum_out=lc2[:, b : b + 1],
                        )
                    else:
                        nc.vector.tensor_reduce(
                            out=lc2[:, b : b + 1], in_=diff[:], axis=AX.X,
                            op=Alu.add, apply_absolute_value=True,
                        )
                if act_tail:
                    lsum = smp.tile([P, BPC], F32, tag="lsum")
                    nc.scalar.activation(
                        out=lsum[:], in_=lc2[:],
                        func=mybir.ActivationFunctionType.Copy,
                        accum_out=losscnt[:, 0:1],
                    )
                else:
                    nc.vector.tensor_reduce(
                        out=losscnt[:, 0:1], in_=lc2[:], axis=AX.X, op=Alu.add
                    )
                csum = smp.tile([P, BPC * JJ], F32, tag="csum")
                nc.scalar.activation(
                    out=csum[:], in_=mselw[:],
                    func=mybir.ActivationFunctionType.Copy,
                    accum_out=losscnt[:, 1:2],
                )
                nc.sync.dma_start(out=out[:, :], in_=losscnt[:])

    nc.compile()
    return nc


_NC = None


def _get_nc():
    global _NC
    if _NC is None:
        _NC = build(init_unused=False)
    return _NC


def make_in_maps(output, rdepth):
    in_maps = []
    for c in range(NCORES):
        sl = slice(c * BPC, (c + 1) * BPC)
        img_c = np.ascontiguousarray(
            output[sl, 0], dtype=np.float32
        ).reshape(BPC * RT, E)
        rdp_c = np.ascontiguousarray(
            rdepth[sl], dtype=np.float32
        ).reshape(BPC * N, 3)
        in_maps.append({"img": img_c, "rdp": rdp_c})
    return in_maps


def combine(results):
    partials = np.stack([r["out"] for r in results])  # [8, 128, 2]
    loss = partials[..., 0].astype(np.float64).sum()
    cnt = partials[..., 1].astype(np.float64).sum()
    val = loss / max(cnt, 1.0) if cnt > 0 else 0.0
    return np.asarray(val, dtype=np.float32)


def run(output, rdepth, **kw):
    res = run_bass_kernel_spmd(
        _get_nc(), make_in_maps(output, rdepth), list(range(NCORES)), **kw
    )
    return combine(res.results), res


def kernel(output, rdepth):
    return run(output, rdepth)[0]


# revision 30
# speedup vs baseline: 1.2677x; 1.0083x over previous
"""DepthLoss kernel for 8 Trainium2 NeuronCores.

reference:
    rows/cols/d = rdepth[...,0/1/2]; mask = d>0
    vals = output[b, 0, rows, cols]
    loss = sum(mask * |vals - d|) / max(count(mask), 1)   (0 if count==0)

Strategy: data-parallel over batch (4 planes/core). Per core:
  - compute pixel index pix = r*W + c on DVE; row id = pix>>6 (int16)
  - dma_gather (SWDGE bulk gather) fetches each sample's 256B image row
  - one-hot select on DVE picks the target element out of each row
  - masked |v - d| partial sums + counts per partition -> [128, 2]
Host combines the 8 cores' partials and does the final divide.

The gather is the hard floor: 65536 descriptors/core over 4 SWDGE
queues at ~8.6ns/descriptor/queue ~= 141us.  Everything else is
arranged to hide under it:
  - idx prep for iteration i+1 runs on DVE while iteration i's gathers
    are in flight (software pipelining across the repeat-slope loop),
    so the Pool engine never waits on DVE at an iteration boundary.
  - the per-batch (row,col,depth) -> csel/mask index chain is computed
    once per iteration at 4-batch width instead of per batch.
  - csel = (cmod+1)*mask compared against iota+1 (avoids the -1 shift).

Index bookkeeping: dma_gather consumes index i from partition i%16,
column i//16 of its idx tile and writes the row to G[i%128, i//128, :].
With row-ids for sample s = 1024*q + 128*(u%8) + u//8 stored at idx
tile [q, u], the gather output G[p, jj] holds sample 128*pi(p) + jj
where pi(p) = 8*(p%16) + p//16.  Loading the per-batch rdepth with a
permuted-partition AP (partition p <- contiguous samples starting at
128*pi(p)) makes d/cmod line up with G with no cross-partition moves.
"""

import numpy as np

import concourse.bacc as bacc
import concourse.mybir as mybir
import concourse.tile as tile
from concourse import library_config
from concourse.bass_utils import run_bass_kernel_spmd

B, N, H, W = 32, 16384, 768, 1024
NCORES = 8
BPC = B // NCORES          # batches (planes) per core = 4
P = 128
PLANE = H * W              # 786432
E = 64                     # gathered row length (f32) = 256 B
RT = PLANE // E            # rows per plane table = 12288
U = N // 16                # idx columns = 1024
JJ = N // P                # samples per partition per batch = 128
HJ = JJ // 2
NCH = 8                    # gather chunks per batch
CI = N // NCH              # idxs per chunk = 1024
CJ = JJ // NCH             # dst cols per chunk = 8
CU = U // NCH              # idx tile cols per chunk = 64
F32 = mybir.dt.float32
BF16 = mybir.dt.bfloat16
I16 = mybir.dt.int16
I32 = mybir.dt.int32
Alu = mybir.AluOpType
AX = mybir.AxisListType


def build(n_iters=1, init_unused=True, nch=NCH, do_select=True, pure=False, rtbw_only=False, gbufs=4, wbufs=2, act_idx=True, wide_tail=False, sel_splits=2, half_sel=False, act_tail=True, dds=16384):
    nc = bacc.Bacc(
        "TRN2", target_bir_lowering=False, debug=False,
        num_swdge_queues=4, dynamic_dma_scratch_size=dds,
    )

    img = nc.dram_tensor("img", [BPC * RT, E], F32, kind="ExternalInput")
    rdp = nc.dram_tensor("rdp", [BPC * N, 3], F32, kind="ExternalInput")
    out = nc.dram_tensor("out", [P, 2], F32, kind="ExternalOutput")

    with tile.TileContext(nc) as tc:
        with (
            tc.tile_pool(name="const", bufs=1) as cst,
            tc.tile_pool(name="rtp", bufs=2) as rtp,
            tc.tile_pool(name="pxp", bufs=1) as pxp,
            tc.tile_pool(name="ixp", bufs=2) as ixp,
            tc.tile_pool(name="big", bufs=gbufs) as big,
            tc.tile_pool(name="wp", bufs=wbufs) as wp,
            tc.tile_pool(name="sm", bufs=2) as smp,
        ):
            nc.gpsimd.load_library(library_config.mlp)
            io_i = cst.tile([P, E], I32, tag="io_i")
            nc.gpsimd.iota(io_i[:], pattern=[[1, E]], channel_multiplier=0)
            iota1 = cst.tile([P, E], F32, tag="iota1")
            nc.vector.tensor_copy(out=iota1[:], in_=io_i[:])
            nc.vector.tensor_scalar(
                out=iota1[:], in0=iota1[:], scalar1=1.0, scalar2=None,
                op0=Alu.add,
            )

            def idx_prep():
                """rt16 load + pix/row computation + idx16 tiles.

                DVE cost ~9us; emitted for iteration i+1 right after
                iteration i's gathers so it overlaps them.  Returns the
                rt16 tile too: the select phase of the SAME iteration
                re-reads it (SBUF->SBUF) to build the gather-layout
                rdepth view instead of re-reading HBM, which would
                contend with the gather stream.
                """
                rt16 = rtp.tile([P, 3 * U], F32, tag="rt16")
                if init_unused:
                    nc.vector.memset(rt16[:], 0)
                for b in range(BPC):
                    nc.sync.dma_start(
                        out=rt16[32 * b : 32 * b + 16, :],
                        in_=rdp[b * N : (b + 1) * N, :].rearrange(
                            "(q u) c -> q (u c)", q=16
                        ),
                    )
                rv16 = rt16[:].rearrange("p (u c) -> p u c", c=3)
                pix = pxp.tile([P, U], F32, tag="pix")
                nc.vector.scalar_tensor_tensor(
                    out=pix[:], in0=rv16[:, :, 0], scalar=float(W),
                    in1=rv16[:, :, 1], op0=Alu.mult, op1=Alu.add,
                )
                pixi = pxp.tile([P, U], I32, tag="pixi")
                if act_idx:
                    nc.scalar.activation(
                        out=pixi[:], in_=pix[:],
                        func=mybir.ActivationFunctionType.Copy,
                    )
                else:
                    nc.vector.tensor_copy(out=pixi[:], in_=pix[:])
                rowi = pxp.tile([P, U], I32, tag="rowi")
                nc.vector.tensor_scalar(
                    out=rowi[:], in0=pixi[:], scalar1=6, scalar2=None,
                    op0=Alu.arith_shift_right,
                )
                idxs = []
                for b in range(BPC):
                    qs = slice(32 * b, 32 * b + 16)
                    # int16 row ids: idx16[q, u] = rowi[32b+q, 128*(u%8)+u//8],
                    # replicated across all 8 gpsimd-core stripes
                    idx16 = ixp.tile([P, U], I16, tag=f"idx16_{b}")
                    oap = (
                        idx16[0:16, :]
                        .rearrange("q (a e) -> q a e", e=8)
                        .transpose([0, 2, 1])
                    )
                    iap = rowi[qs, :].rearrange("q (e a) -> q e a", e=8)
                    if act_idx:
                        nc.scalar.activation(
                            out=oap, in_=iap,
                            func=mybir.ActivationFunctionType.Copy,
                        )
                    else:
                        nc.vector.tensor_copy(out=oap, in_=iap)
                    nc.sync.dma_start(out=idx16[16:32, :], in_=idx16[0:16, :])
                    nc.sync.dma_start(out=idx16[32:64, :], in_=idx16[0:32, :])
                    nc.sync.dma_start(out=idx16[64:128, :], in_=idx16[0:64, :])
                    idxs.append(idx16)
                return idxs, rt16

            idxs, rt16c = idx_prep()
            for it in range(n_iters):
                # per-batch rdepth in gather layout, all batches wide:
                # rtbw[16a+q, b, t, c] = rdepth[b, 1024q+128a+t, c]
                #                      = rt16[32b+q, 384a+3t+c]
                # (pure SBUF->SBUF relayout of rt16; no HBM traffic, so
                #  it does not disturb the gather stream.  Issued from
                #  the otherwise-idle ACT queue.)
                rtbw = smp.tile([P, BPC * 3 * JJ], F32, tag="rtbw")
                if pure and not rtbw_only:
                    nc.vector.memset(rtbw[:], 0)
                for b in range(BPC if (not pure or rtbw_only) else 0):
                    for a in range(8):
                        nc.scalar.dma_start(
                            out=rtbw[
                                16 * a : 16 * (a + 1),
                                b * 3 * JJ : (b + 1) * 3 * JJ,
                            ],
                            in_=rt16c[
                                32 * b : 32 * b + 16,
                                384 * a : 384 * (a + 1),
                            ],
                        )

                # gathers: G[p, jj, :] = img row of sample 128*pi(p)+jj
                # (chunked so each SWDGE op fits the descriptor ring and
                #  rotates the 4 queues)
                gs = {}
                ci = N // nch
                cj = JJ // nch
                cu = U // nch
                hk = nch // 2          # chunks per half
                for b in range(BPC):
                    for h in range(2):
                        g = big.tile([P, HJ * E], F32, tag="G")
                        g3 = g[:].rearrange("p (j e) -> p j e", e=E)
                        gs[(b, h)] = g3
                        for kk in range(hk):
                            k = h * hk + kk
                            nc.gpsimd.dma_gather(
                                g3[:, kk * cj : (kk + 1) * cj, :],
                                img[b * RT : (b + 1) * RT, :],
                                idxs[b][:, k * cu : (k + 1) * cu],
                                ci,
                                ci,
                                E,
                                single_packet=False,
                                queue_num=k % 4,
                            )

                # wide select-index chain (all 4 batches at once);
                # emitted BEFORE next-iter idx prep so the first select
                # is never gated on the DVE finishing idx work
                rv = rtbw[:].rearrange("p (b t c) -> p b t c", b=BPC, c=3)
                pixw = smp.tile([P, BPC * JJ], F32, tag="pixw")
                pw3 = pixw[:].rearrange("p (b t) -> p b t", b=BPC)
                nc.vector.scalar_tensor_tensor(
                    out=pw3, in0=rv[:, :, :, 0], scalar=float(W),
                    in1=rv[:, :, :, 1], op0=Alu.mult, op1=Alu.add,
                )
                pixwi = smp.tile([P, BPC * JJ], I32, tag="pixwi")
                nc.vector.tensor_copy(out=pixwi[:], in_=pixw[:])
                cmw = smp.tile([P, BPC * JJ], I32, tag="cmw")
                nc.vector.tensor_scalar(
                    out=cmw[:], in0=pixwi[:], scalar1=E - 1, scalar2=None,
                    op0=Alu.bitwise_and,
                )
                cselw = smp.tile([P, BPC * JJ], F32, tag="cselw")
                nc.scalar.activation(
                    out=cselw[:], in_=cmw[:],
                    func=mybir.ActivationFunctionType.Copy,
                )
                mselw = smp.tile([P, BPC * JJ], F32, tag="mselw")
                mw3 = mselw[:].rearrange("p (b t) -> p b t", b=BPC)
                nc.scalar.activation(
                    out=mw3, in_=rv[:, :, :, 2],
                    func=mybir.ActivationFunctionType.Sign,
                )
                nc.scalar.activation(
                    out=mselw[:], in_=mselw[:],
                    func=mybir.ActivationFunctionType.Relu,
                )
                # csel = (cmod+1)*mask; invalid samples select nothing
                # (compared against iota+1 which is >= 1)
                nc.vector.scalar_tensor_tensor(
                    out=cselw[:], in0=cselw[:], scalar=1.0, in1=mselw[:],
                    op0=Alu.add, op1=Alu.mult,
                )
                cs3 = cselw[:].rearrange("p (b t) -> p b t", b=BPC)
                ms3 = mselw[:].rearrange("p (b t) -> p b t", b=BPC)

                # idx prep for the NEXT iteration: overlaps this
                # iteration's gathers
                if it + 1 < n_iters:
                    idxs, rt16c = idx_prep()

                vsels = []
                sj = JJ // sel_splits       # samples per select split
                sh = sel_splits // 2        # splits per gather half
                for b in range(BPC if do_select else 0):
                    vsel = smp.tile([P, JJ], F32, tag="vsel")
                    vsels.append(vsel)
                    if half_sel:
                        nc.vector.memset(vsel[:, HJ:], 0)
                    # one-hot select: W = (iota+1 == csel); v = sum(G*W)
                    # split so each piece's mul starts as soon as its
                    # gather chunks land (subtile deps)
                    for q in range(sel_splits if not half_sel else 1):
                        js = slice(q * sj, (q + 1) * sj)
                        h = q // sh
                        jg = slice((q % sh) * sj, (q % sh + 1) * sj)
                        w = wp.tile([P, sj * E], F32, tag="W")
                        w3 = w[:].rearrange("p (j e) -> p j e", e=E)
                        nc.vector.tensor_tensor(
                            out=w3,
                            in0=iota1[:].unsqueeze(1).to_broadcast(
                                [P, sj, E]
                            ),
                            in1=cs3[:, b, js].unsqueeze(2).to_broadcast(
                                [P, sj, E]
                            ),
                            op=Alu.is_equal,
                        )
                        nc.vector.tensor_tensor(
                            out=w3, in0=gs[(b, h)][:, jg, :], in1=w3,
                            op=Alu.mult,
                        )
                        nc.vector.tensor_reduce(
                            out=vsels[b][:, js], in_=w3, axis=AX.X, op=Alu.add
                        )
                losscnt = smp.tile([P, 2], F32, tag="losscnt")
                lc2 = smp.tile([P, BPC], F32, tag="lc2")
                if not do_select:
                    nc.vector.memset(lc2[:], 0)
                for b in range(BPC if do_select else 0):
                    # masked |v - d|
                    diff = smp.tile([P, JJ], F32, tag="diff")
                    nc.vector.tensor_tensor(
                        out=diff[:], in0=vsels[b][:], in1=rv[:, b, :, 2],
                        op=Alu.subtract,
                    )
                    nc.vector.tensor_tensor(
                        out=diff[:], in0=diff[:], in1=ms3[:, b, :],
                        op=Alu.mult,
                    )
                    if act_tail:
                        # |.| + free-dim accumulate in one ACT op
                        absd = smp.tile([P, JJ], F32, tag="absd")
                        nc.scalar.activation(
                            out=absd[:], in_=diff[:],
                            func=mybir.ActivationFunctionType.Abs,
                            accum_out=lc2[:, b : b + 1],
                        )
                    else:
                        nc.vector.tensor_reduce(
                            out=lc2[:, b : b + 1], in_=diff[:], axis=AX.X,
                            op=Alu.add, apply_absolute_value=True,
                        )
                if act_tail:
                    lsum = smp.tile([P, BPC], F32, tag="lsum")
                    nc.scalar.activation(
                        out=lsum[:], in_=lc2[:],
                        func=mybir.ActivationFunctionType.Copy,
                        accum_out=losscnt[:, 0:1],
                    )
                else:
                    nc.vector.tensor_reduce(
                        out=losscnt[:, 0:1], in_=lc2[:], axis=AX.X, op=Alu.add
                    )
                csum = smp.tile([P, BPC * JJ], F32, tag="csum")
                nc.scalar.activation(
                    out=csum[:], in_=mselw[:],
                    func=mybir.ActivationFunctionType.Copy,
                    accum_out=losscnt[:, 1:2],
                )
                nc.sync.dma_start(out=out[:, :], in_=losscnt[:])

    nc.compile()
    return nc


_NC = None


def _get_nc():
    global _NC
    if _NC is None:
        _NC = build(init_unused=False)
    return _NC


def make_in_maps(output, rdepth):
    in_maps = []
    for c in range(NCORES):
        sl = slice(c * BPC, (c + 1) * BPC)
        img_c = np.ascontiguousarray(
            output[sl, 0], dtype=np.float32
        ).reshape(BPC * RT, E)
        rdp_c = np.ascontiguousarray(
            rdepth[sl], dtype=np.float32
        ).reshape(BPC * N, 3)
        in_maps.append({"img": img_c, "rdp": rdp_c})
    return in_maps


def combine(results):
    partials = np.stack([r["out"] for r in results])  # [8, 128, 2]
    loss = partials[..., 0].astype(np.float64).sum()
    cnt = partials[..., 1].astype(np.float64).sum()
    val = loss / max(cnt, 1.0) if cnt > 0 else 0.0
    return np.asarray(val, dtype=np.float32)


def run(output, rdepth, **kw):
    res = run_bass_kernel_spmd(
        _get_nc(), make_in_maps(output, rdepth), list(range(NCORES)), **kw
    )
    return combine(res.results), res


def kernel(output, rdepth):
    return run(output, rdepth)[0]
